# revision 1
# baseline (speedup 1.0000x reference)
"""GroupedQueryAttention TRN2 kernel.

Sharding: 8 cores = (batch b in 0..1) x (kv-group g in 0..3). Each core
computes, for its batch and its kv head group (1 kv head, 4 query heads):
  q = x[b] @ Wq[:, g*256:(g+1)*256]          [2048, 256]
  k = x[b] @ Wkv[:, g*64:(g+1)*64]           [2048, 64]
  v = x[b] @ Wkv[:, 256+g*64:256+(g+1)*64]   [2048, 64]
  causal softmax attention per head          [2048, 256]
  partial_out = attn_out @ Wo[g*256:(g+1)*256, :]   [2048, 1024]
Host sums the 4 partials per batch (row-parallel Wo).

On-chip layout is fully transposed (feature dims on partitions) so no data
transposes are needed on the critical path:
  - scores are computed as S^T[tk, tq] (lhsT = kT tile, rhs = qT slice)
  - softmax denominators come from a ones-column appended to v (lhsT of the
    attn@v matmul), division is applied between two cheap PE transposes.
  - the causal mask is ADDED to score PSUM via a bf16 identity matmul
    (out += I.T @ mask_tile), so no vector-engine masking is needed.
Matmuls run in float32r (full PE speed, ~tf32 accuracy); attention
probabilities in bf16.
"""

import numpy as np
import ml_dtypes

import concourse.bass as bass
import concourse.mybir as mybir
import concourse.tile as tile
from concourse import bacc
from concourse.bass_utils import run_bass_kernel_spmd

B, T, DIM = 2, 2048, 1024
NH, NKV = 16, 4
HD = DIM // NH  # 64
R = NH // NKV  # 4
HQ = R * HD  # 256 query cols per core
NJ = T // 128  # 16 key tiles
NCH = T // 512  # 4 query chunks of 512
NEG = -30000.0

F32R = mybir.dt.float32r
BF16 = mybir.dt.bfloat16
F32 = mybir.dt.float32

_CACHED_NC = None


def _cfg(c, j):
    """Per (chunk, key-tile): (tq start within chunk, width, mask kind)."""
    m = j - 4 * c
    if m < 0:
        return 0, 512, None
    if m == 0:
        return 0, 512, "lt"
    if m == 1:
        return 128, 384, "lt"
    if m == 2:
        return 256, 256, "lt"
    return 256, 256, "m3"


def build_nc():
    nc = bacc.Bacc()
    xT = nc.declare_dram_parameter("xT", [DIM, T], F32R, isOutput=False)
    wq = nc.declare_dram_parameter("wq", [DIM, HQ], F32R, isOutput=False)
    # [k|v|v|k] columns: M-tile A -> [kT;vT] rows, M-tile B -> [vT;kT] rows
    wkv2 = nc.declare_dram_parameter("wkv2", [DIM, 256], F32R, isOutput=False)
    wo = nc.declare_dram_parameter("wo", [HQ, DIM], F32R, isOutput=False)
    mlt = nc.declare_dram_parameter("mlt", [128, 128], BF16, isOutput=False)
    mm3 = nc.declare_dram_parameter("mm3", [128, 256], BF16, isOutput=False)
    idf = nc.declare_dram_parameter("idf", [128, 128], F32, isOutput=False)
    idb = nc.declare_dram_parameter("idb", [128, 128], BF16, isOutput=False)
    out = nc.declare_dram_parameter("out", [T, DIM], F32, isOutput=True)

    with tile.TileContext(nc) as tc:
        with (
            tc.tile_pool(name="persist", bufs=1) as pp,
            tc.tile_pool(name="vaug_p", bufs=NJ) as vp,
            tc.tile_pool(name="pt_p", bufs=3) as ptp,
            tc.tile_pool(name="avd_p", bufs=3) as adp,
            tc.tile_pool(name="out_p", bufs=3) as op,
            tc.tile_pool(name="small", bufs=4) as sp,
            tc.tile_pool(name="avs_p", bufs=10) as avsp,
            tc.tile_pool(name="ps_s", bufs=2, space="PSUM") as pss,
            tc.tile_pool(name="ps_av", bufs=2, space="PSUM") as psav,
            tc.tile_pool(name="ps_m", bufs=2, space="PSUM") as psm,
        ):
            # ---- constants / weights ----
            # fp32r is not a legal transpose-mode dtype (walrus
            # s3d3_mm_fp32r_restrictions), so transposes run in fp32 or bf16
            # with host-provided identities.
            ident_f = pp.tile([128, 128], F32, tag="ident_f")
            nc.sync.dma_start(out=ident_f, in_=idf[:, :])
            ident_b = pp.tile([128, 128], BF16, tag="ident_b")
            nc.sync.dma_start(out=ident_b, in_=idb[:, :])
            mlt_sb = pp.tile([128, 128], BF16, tag="mlt")
            nc.sync.dma_start(out=mlt_sb, in_=mlt[:, :])
            mm3_sb = pp.tile([128, 256], BF16, tag="mm3")
            nc.sync.dma_start(out=mm3_sb, in_=mm3[:, :])
            wq_sb = pp.tile([128, 8, HQ], F32R, tag="wq")
            nc.sync.dma_start(out=wq_sb, in_=wq.rearrange("(k p) m -> p k m", p=128))
            wkv_sb = pp.tile([128, 8, 256], F32R, tag="wkv")
            nc.sync.dma_start(out=wkv_sb, in_=wkv2.rearrange("(k p) m -> p k m", p=128))
            wo_sb = pp.tile([128, 2, DIM], F32R, tag="wo")
            nc.sync.dma_start(out=wo_sb, in_=wo.rearrange("(c p) n -> p c n", p=128))
            xt_sb = pp.tile([128, 8, T], F32R, tag="xt")
            for kd in range(8):
                nc.sync.dma_start(
                    out=xt_sb[:, kd, :], in_=xT[kd * 128 : (kd + 1) * 128, :]
                )

            qt_sb = pp.tile([128, 2, T], F32R, tag="qt")
            kva_sb = pp.tile([128, T], F32R, tag="kva")  # rows 0:64 = kT
            kvb_sb = pp.tile([128, T], F32R, tag="kvb")  # rows 64:128 = kT dup
            vtb_sb = pp.tile([64, T], BF16, tag="vtb")  # vT in bf16
            avt01 = pp.tile([128, T], F32R, tag="avt01")
            avt23 = pp.tile([128, T], F32R, tag="avt23")

            eng = [0]

            def cp(dst, src):
                # alternate drain engine to balance ACT/DVE load
                if eng[0] % 2 == 0:
                    nc.scalar.copy(dst, src)
                else:
                    nc.vector.tensor_copy(out=dst, in_=src)
                eng[0] += 1

            # ---- qkv projections (xT stationary, weights as lhsT) ----
            vaug = [None] * NJ

            def qkv_chunk(n):
                cols = slice(n * 512, (n + 1) * 512)
                for m in range(2):
                    pq = psm.tile([128, 512], F32, tag="m")
                    for kd in range(8):
                        nc.tensor.matmul(
                            pq,
                            lhsT=wq_sb[:, kd, m * 128 : (m + 1) * 128],
                            rhs=xt_sb[:, kd, cols],
                            start=(kd == 0),
                            stop=(kd == 7),
                        )
                    cp(qt_sb[:, m, cols], pq)
                for mt in range(2):
                    pkv = psm.tile([128, 512], F32, tag="m")
                    for kd in range(8):
                        nc.tensor.matmul(
                            pkv,
                            lhsT=wkv_sb[:, kd, mt * 128 : (mt + 1) * 128],
                            rhs=xt_sb[:, kd, cols],
                            start=(kd == 0),
                            stop=(kd == 7),
                        )
                    if mt == 0:  # [kT; vT]: keep kT for even heads
                        cp(kva_sb[0:64, cols], pkv[0:64, :])
                    else:  # [vT; kT]: vT (bf16) + kT dup for odd heads
                        nc.vector.tensor_copy(out=vtb_sb[:, cols], in_=pkv[0:64, :])
                        cp(kvb_sb[64:128, cols], pkv[64:128, :])
                for tt in range(4):
                    j = n * 4 + tt
                    ptr = psm.tile([128, 64], BF16, tag="m")
                    nc.tensor.transpose(
                        ptr,
                        in_=vtb_sb[0:64, j * 128 : (j + 1) * 128],
                        identity=ident_b[0:64, 0:64],
                    )
                    va = vp.tile([128, 65], BF16, tag="vaug")
                    nc.vector.tensor_copy(out=va[:, 0:64], in_=ptr)
                    nc.gpsimd.memset(va[:, 64:65], 1.0)
                    vaug[j] = va

            # ---- attention + output projection ----
            def post_stage1(h, c, av):
                """drain av psum, transpose slabs + denominators, scale."""
                # single drain of the whole av psum (rows 0:64 = av^T slabs,
                # row 64 = softmax denominators); DVE so ACT stays on exp
                avd = adp.tile([65, 512], F32, tag="avd")
                nc.vector.tensor_copy(out=avd, in_=av)
                # one bank: av^T tiles in cols 0:256, l^T in cols 256:260
                pt1 = psm.tile([128, 320], F32, tag="m")
                for tt in range(4):
                    nc.tensor.matmul(
                        pt1[:, tt * 64 : (tt + 1) * 64],
                        lhsT=avd[0:64, tt * 128 : (tt + 1) * 128],
                        rhs=ident_f[0:64, 0:64],
                        is_transpose=True,
                        start=(tt == 0),
                        stop=False,
                    )
                for tt in range(4):
                    nc.tensor.matmul(
                        pt1[:, 256 + tt : 257 + tt],
                        lhsT=avd[64:65, tt * 128 : (tt + 1) * 128],
                        rhs=ident_f[64:65, 64:65],
                        is_transpose=True,
                        start=False,
                        stop=(tt == 3),
                    )
                rt = sp.tile([128, 4], F32, tag="rt")
                nc.vector.reciprocal(out=rt, in_=pt1[:, 256:260])
                avss = []
                for tt in range(4):
                    avs = avsp.tile([128, 64], F32, tag="avs")
                    nc.vector.tensor_scalar_mul(
                        out=avs,
                        in0=pt1[:, tt * 64 : (tt + 1) * 64],
                        scalar1=rt[:, tt : tt + 1],
                    )
                    avss.append(avs)
                return h, c, avss

            def post_stage2(h, c, avss):
                """transpose scaled slabs back and pack into avT tiles."""
                hb = (h % 2) * 64
                avt = avt01 if h < 2 else avt23
                # transpose outputs must start at PSUM partition 0; odd heads
                # reach avt partitions 64:128 via an SBUF->SBUF DMA bounce.
                pt2 = psm.tile([64, 512], F32, tag="m")
                for tt in range(4):
                    nc.tensor.matmul(
                        pt2[:, tt * 128 : (tt + 1) * 128],
                        lhsT=avss[tt],
                        rhs=ident_f,
                        is_transpose=True,
                        start=(tt == 0),
                        stop=(tt == 3),
                    )
                if hb == 0:
                    nc.vector.tensor_copy(
                        out=avt[0:64, c * 512 : (c + 1) * 512], in_=pt2
                    )
                else:
                    avh = adp.tile([64, 512], F32R, tag="avh")
                    nc.vector.tensor_copy(out=avh, in_=pt2)
                    nc.sync.dma_start(
                        out=avt[64:128, c * 512 : (c + 1) * 512], in_=avh
                    )

            def attn_c(c):
                q1 = []  # (h, c, av) awaiting stage1
                q2 = []  # (h, c, avss) awaiting stage2
                for h in range(4):
                    hb = (h % 2) * 64
                    k_sb = kva_sb if h % 2 == 0 else kvb_sb
                    jmax = 4 * c + 3
                    av = psav.tile([65, 512], F32, tag="av")
                    for gi in range(2 * c + 2):
                        ja, jb = 2 * gi, 2 * gi + 1
                        sa_a, wa, mk_a = _cfg(c, ja)
                        sa_b, wb, mk_b = _cfg(c, jb)
                        spt = pss.tile([128, 1024], F32, tag="s")
                        ptt = ptp.tile([128, 1024], BF16, tag="pt")
                        bank_shared = wa + wb <= 512  # only the (m2, m3) pair
                        for (j, sa, w, mk, pa) in (
                            (ja, sa_a, wa, mk_a, 0),
                            (jb, sa_b, wb, mk_b, wa),
                        ):
                            if bank_shared:
                                s_start, s_stop = (j == ja), False
                                m_stop = j == jb
                            else:
                                s_start = True
                                s_stop = mk is None
                                m_stop = True
                            nc.tensor.matmul(
                                spt[:, pa : pa + w],
                                lhsT=k_sb[hb : hb + 64, j * 128 : (j + 1) * 128],
                                rhs=qt_sb[
                                    hb : hb + 64,
                                    h // 2,
                                    c * 512 + sa : c * 512 + sa + w,
                                ],
                                start=s_start,
                                stop=s_stop,
                            )
                            if mk is not None:
                                mask_ap = mlt_sb if mk == "lt" else mm3_sb
                                mw = 128 if mk == "lt" else 256
                                nc.tensor.matmul(
                                    spt[:, pa : pa + mw],
                                    lhsT=ident_b,
                                    rhs=mask_ap,
                                    start=False,
                                    stop=m_stop,
                                )
                        W = wa + wb
                        nc.scalar.activation(
                            out=ptt[:, 0:W],
                            in_=spt[:, 0:W],
                            func=mybir.ActivationFunctionType.Exp,
                            scale=0.125,
                        )
                        for (j, sa, w, pa) in (
                            (ja, sa_a, wa, 0),
                            (jb, sa_b, wb, wa),
                        ):
                            nc.tensor.matmul(
                                av[:, sa : sa + w],
                                lhsT=vaug[j][:, 0:65],
                                rhs=ptt[:, pa : pa + w],
                                start=(j == 0),
                                stop=(j == jmax),
                            )
                    if q2:
                        post_stage2(*q2.pop(0))
                    if q1:
                        q2.append(post_stage1(*q1.pop(0)))
                    q1.append((h, c, av))
                while q1 or q2:
                    if q2:
                        post_stage2(*q2.pop(0))
                    if q1:
                        q2.append(post_stage1(*q1.pop(0)))

            def outproj(c):
                # output projection for chunk c's 4 row tiles
                for tt in range(4):
                    trow = c * 4 + tt
                    tcols = slice(trow * 128, (trow + 1) * 128)
                    osb = op.tile([128, DIM], F32, tag="osb")
                    for dch in range(2):
                        dcols = slice(dch * 512, (dch + 1) * 512)
                        po = psm.tile([128, 512], F32, tag="m")
                        nc.tensor.matmul(
                            po,
                            lhsT=avt01[:, tcols],
                            rhs=wo_sb[:, 0, dcols],
                            start=True,
                            stop=False,
                        )
                        nc.tensor.matmul(
                            po,
                            lhsT=avt23[:, tcols],
                            rhs=wo_sb[:, 1, dcols],
                            start=False,
                            stop=True,
                        )
                        nc.vector.tensor_copy(out=osb[:, dcols], in_=po)
                    nc.sync.dma_start(
                        out=out[trow * 128 : (trow + 1) * 128, :], in_=osb
                    )

            # interleave: qkv of chunk c+1 and out-proj of chunk c-1 are
            # emitted around attn of chunk c so PE always has independent
            # work while ACT drains the exp queue.
            qkv_chunk(0)
            for c in range(NCH):
                if c + 1 < NCH:
                    qkv_chunk(c + 1)
                if c >= 1:
                    outproj(c - 1)
                attn_c(c)
            outproj(NCH - 1)

    nc.compile()
    return nc


def _masks():
    idx = np.arange(128)
    lt = np.where(idx[:, None] > idx[None, :], NEG, 0.0).astype(ml_dtypes.bfloat16)
    m3 = np.concatenate(
        [np.full((128, 128), NEG, dtype=ml_dtypes.bfloat16), lt], axis=1
    )
    return lt, m3


def make_in_maps(x, Wq, Wkv, Wo):
    x = np.asarray(x, dtype=np.float32)
    Wq = np.asarray(Wq, dtype=np.float32)
    Wkv = np.asarray(Wkv, dtype=np.float32)
    Wo = np.asarray(Wo, dtype=np.float32)
    mlt_np, mm3_np = _masks()
    in_maps = []
    for core in range(8):
        b, g = divmod(core, NKV)
        k_loc = Wkv[:, g * HD : (g + 1) * HD]
        v_loc = Wkv[:, NKV * HD + g * HD : NKV * HD + (g + 1) * HD]
        in_maps.append(
            {
                "xT": np.ascontiguousarray(x[b].T),
                "wq": np.ascontiguousarray(Wq[:, g * HQ : (g + 1) * HQ]),
                "wkv2": np.ascontiguousarray(
                    np.concatenate([k_loc, v_loc, v_loc, k_loc], axis=1)
                ),
                "wo": np.ascontiguousarray(Wo[g * HQ : (g + 1) * HQ, :]),
                "mlt": mlt_np,
                "mm3": mm3_np,
                "idf": np.eye(128, dtype=np.float32),
                "idb": np.eye(128, dtype=ml_dtypes.bfloat16),
            }
        )
    return in_maps


def gather(results):
    outs = [results[i]["out"].astype(np.float64) for i in range(8)]
    return np.stack(
        [
            outs[0] + outs[1] + outs[2] + outs[3],
            outs[4] + outs[5] + outs[6] + outs[7],
        ]
    ).astype(np.float32)


def kernel(x, Wq, Wkv, Wo):
    global _CACHED_NC
    if _CACHED_NC is None:
        _CACHED_NC = build_nc()
    in_maps = make_in_maps(x, Wq, Wkv, Wo)
    res = run_bass_kernel_spmd(_CACHED_NC, in_maps, list(range(8)))
    return gather(res.results)

